# revision 22
# baseline (speedup 1.0000x reference)
"""MHSA over 32 independent 512-token segments, segment-parallel across 8
NeuronCores (4 segments / 2048 tokens per core, zero cross-core traffic).

All operands bf16 (converted host-side); matmul accumulation f32 in PSUM.
w_attn / w_proj are SBUF-resident for the whole kernel (loaded once).

Per core, per segment s:
  x^T        via PE transpose of bf16 x                   8x [128, 512]
  Q^T,K^T    = (W^T x^T) : lhsT=W chunks, rhs=x^T         16x [128, 512]
  V          = x @ Wv    : lhsT=x^T chunks, rhs=W         natural [tok, 1024]
  S^T        = K Q^T per head (K=64)                      [512k, 512q] psum
  A^T        = exp(S^T/8)  bf16 (no max-sub: |S/8|<~6)    [512, 512]
  O          = A^T.T @ [V|1] per q-chunk: out [128q, 65]  (col 64 = rowsum Z)
  Y          = O[:, 0:64] * (1/Z)  per-partition scalar   (DVE, no PE)
  Y^T        via PE transpose (bf16)
  out        = Y^T.T @ Wproj

Schedule: phase C (attention) of segment s is interleaved with x-transpose /
QKV matmuls of segment s+1 (and, for the last segment, with the previous
segment's projection) so the PE never waits on the Act-bound exp chain.

PSUM banks: 0-3 S^T staging (head-parity pairs), 4-5 A.V outputs,
6 QKV-unit accumulation, 7 (bf16) transpose staging halves.
"""

import numpy as np
import ml_dtypes

import concourse.bass as bass
import concourse.mybir as mybir
import concourse.tile as tile
from concourse.bass_utils import run_bass_kernel_spmd

F32 = mybir.dt.float32
BF16 = mybir.dt.bfloat16
EXP = mybir.ActivationFunctionType.Exp

T, C, H, HD = 16384, 1024, 16, 64
NCORES = 8
TOK = T // NCORES          # 2048 tokens per core
SEG = 512                  # tokens per segment
NSEG = TOK // SEG          # 4 segments per core
SCALE = 1.0 / np.sqrt(HD)  # folded into exp()

# filler unit PE costs (ns) for pacing phase-C interleave
COST_A, COST_B, COST_D = 220, 1710, 1750
SLOT_FILL = 1050           # target filler ns per head slot


def _split_multi_waits(nc):
    """Drop provably-satisfied same-engine waits, then move any remaining
    extra sync waits onto same-engine NoOps (1-wait ISA limit).

    A wait on the instruction's own engine's semaphore with wait_value <=
    (count of prior same-engine updates to that semaphore) is guaranteed by
    in-order engine execution; keeping it (or worse, splitting it onto a
    NoOp) stalls the sequencer until the previous instruction completes and
    the semaphore propagates, serializing back-to-back engine work.
    """
    for fn in nc.m.functions:
        for bb in fn.blocks:
            out = []
            for inst in bb.instructions:
                si = inst.sync_info
                eng = str(inst.engine).split(".")[-1]
                if si is not None and si.on_wait and len(si.on_wait) > 1:
                    # keep the same-engine wait (if any) on the instruction —
                    # it resolves at the wait-queue head with minimal delay,
                    # whereas a NoOp wait blocks the sequencer outright
                    waits = sorted(
                        si.on_wait,
                        key=lambda w: (
                            isinstance(w.ant_name, str)
                            and w.ant_name.split("_")[0] == eng
                        ),
                    )
                    for j, w in enumerate(waits[:-1]):
                        nop = mybir.InstNoOp(name=f"{inst.name}-wsp{j}")
                        nop.engine = inst.engine
                        nop.sync_info = mybir.SyncInfo(on_wait=[w], on_update=[])
                        out.append(nop)
                    inst.sync_info = mybir.SyncInfo(
                        on_wait=[waits[-1]], on_update=list(si.on_update)
                    )
                out.append(inst)
            bb.instructions = out


def _build():
    nc = bass.Bass("TRN2", target_bir_lowering=False, debug=False)
    x = nc.dram_tensor("x_sh", [TOK, C], BF16, kind="ExternalInput").ap()
    wa = nc.dram_tensor("w_attn", [C, 3 * C], BF16, kind="ExternalInput").ap()
    wp = nc.dram_tensor("w_proj", [C, C], BF16, kind="ExternalInput").ap()
    out = nc.dram_tensor("out", [TOK, C], F32, kind="ExternalOutput").ap()

    ident_d = nc.inline_tensor(
        np.eye(128, dtype=np.float32).astype(ml_dtypes.bfloat16), "ident_c"
    ).ap()

    with tile.TileContext(nc) as tc:
        with (
            tc.tile_pool(name="const", bufs=1) as cpool,
            tc.tile_pool(name="wres", bufs=1) as wres,
            tc.tile_pool(name="stream", bufs=1) as stream,
            tc.tile_pool(name="work", bufs=1) as work,
            tc.tile_pool(name="ps", bufs=1, space="PSUM") as pspool,
        ):
            # ---- PSUM layout (allocated in order, bank = 512 f32)
            psA = pspool.tile([128, 2048], F32, tag="psA", name="psA")   # banks 0-3
            av = [pspool.tile([128, 512], F32, tag=f"av{i}", name=f"av{i}")
                  for i in range(2)]                                     # banks 4-5
            bu0 = pspool.tile([128, 512], F32, tag="bu0", name="bu0")    # bank 6
            trb = pspool.tile([128, 1024], BF16, tag="trb", name="trb")  # bank 7
            atr = trb[:, 0:512]
            dtr = trb[:, 512:1024]

            def sbank(i):
                return psA[:, 512 * i:512 * (i + 1)]

            ident = cpool.tile([128, 128], BF16, tag="ident", name="ident")
            nc.sync.dma_start(ident[:], ident_d[:, :])

            # ---- resident weights (bf16, loaded once), wa split per QKV group
            wa_sb = [wres.tile([128, 3 * C], BF16, tag=f"wa{cc}", name=f"wa{cc}")
                     for cc in range(8)]
            wp_sb = [wres.tile([128, C], BF16, tag=f"wp{cc}", name=f"wp{cc}")
                     for cc in range(8)]

            # ---- per-segment working tiles, double-buffered by parity
            xn = [[stream.tile([128, C], BF16, tag=f"xn{p}_{qt}", name=f"xn{p}_{qt}")
                   for qt in range(4)] for p in range(2)]
            xT = [[work.tile([128, SEG], BF16, tag=f"xT{p}_{cc}", name=f"xT{p}_{cc}")
                   for cc in range(8)] for p in range(2)]
            qkt = [[work.tile([128, SEG], BF16, tag=f"qkt{p}_{m}", name=f"qkt{p}_{m}")
                    for m in range(16)] for p in range(2)]
            vp = [[work.tile([128, 16 * 66], BF16, tag=f"vp{p}_{qt}", name=f"vp{p}_{qt}")
                   for qt in range(4)] for p in range(2)]
            ytp = [[work.tile([128, C], BF16, tag=f"ytp{p}_{qc}", name=f"ytp{p}_{qc}")
                    for qc in range(4)] for p in range(2)]
            yt_all = work.tile([128, 4096], BF16, tag="yt", name="yt")

            # ones columns of vp (col 64 of each 66-stride head block) persist
            for p in range(2):
                for qt in range(4):
                    nc.vector.memset(
                        vp[p][qt].rearrange("p (h w) -> p h w", w=66)[:, :, 64:65], 1.0
                    )

            # ---------- emission helpers ----------
            def dma_xn(s):
                p = s % 2
                for qt in range(4):
                    nc.sync.dma_start(
                        xn[p][qt][:],
                        x[s * SEG + qt * 128: s * SEG + (qt + 1) * 128, :],
                    )

            def a_unit(s, cc):
                # x^T for channel chunk cc: 4 transposes into bf16 staging
                p = s % 2
                st = atr if cc % 2 == 0 else dtr
                for qt in range(4):
                    nc.tensor.transpose(
                        st[:, qt * 128:(qt + 1) * 128],
                        xn[p][qt][:, cc * 128:(cc + 1) * 128], ident[:],
                    )
                nc.vector.tensor_copy(xT[p][cc][:], st[:, :])

            def qk_unit(s, u, b):
                # Q^T/K^T channel chunk: g = u//8 (0=Q,1=K), m = u%8
                p = s % 2
                g, m = u // 8, u % 8
                for cc in range(8):
                    nc.tensor.matmul(
                        b[:, :],
                        wa_sb[cc][:, g * C + m * 128: g * C + (m + 1) * 128],
                        xT[p][cc][:],
                        start=(cc == 0), stop=(cc == 7),
                    )
                nc.vector.tensor_copy(qkt[p][g * 8 + m][:], b[:, :])

            def v_unit(s, u, b):
                # V tok-chunk qt = u//2, channel half vn = u%2
                p = s % 2
                qt, vn = u // 2, u % 2
                for cc in range(8):
                    nc.tensor.matmul(
                        b[:, :],
                        xT[p][cc][:, qt * 128:(qt + 1) * 128],
                        wa_sb[cc][:, 2 * C + vn * 512: 2 * C + (vn + 1) * 512],
                        start=(cc == 0), stop=(cc == 7),
                    )
                nc.vector.tensor_copy(
                    vp[p][qt].rearrange("p (h w) -> p h w", w=66)[:, vn * 8:(vn + 1) * 8, 0:64],
                    b.rearrange("p (h w) -> p h w", w=64),
                )

            at0s = {}

            def head_s(s, h):
                # S^T chunks into banks 0-3 + two exp halves -> at0 (bf16)
                p = s % 2
                qk_q = qkt[p][h // 2]
                qk_k = qkt[p][8 + h // 2]
                r0 = (h % 2) * 64
                at0 = at0s[h % 2] = work.tile([128, 2048], BF16, tag="at0",
                                              bufs=2, name=f"at0_{s}_{h}")
                for j in range(2):
                    for i in range(2):
                        kt = j * 2 + i
                        nc.tensor.matmul(
                            sbank(kt),
                            qk_k[r0:r0 + 64, kt * 128:(kt + 1) * 128],
                            qk_q[r0:r0 + 64, :], start=True, stop=True,
                        )
                    nc.scalar.activation(
                        at0[:, j * 1024:(j + 1) * 1024],
                        psA[:, j * 1024:(j + 1) * 1024], EXP, scale=SCALE,
                    )

            def head_av(s, h):
                p = s % 2
                at0 = at0s[h % 2]
                a = av[h % 2]
                for qc in range(4):
                    for kt in range(4):
                        nc.tensor.matmul(
                            a[:, qc * 128: qc * 128 + 65],
                            at0[:, kt * 512 + qc * 128: kt * 512 + (qc + 1) * 128],
                            vp[p][kt][:, 66 * h: 66 * h + 65],
                            start=(kt == 0), stop=(kt == 3),
                        )
                zrec = work.tile([128, 4], F32, tag="zrec", bufs=2,
                                 name=f"zrec_{s}_{h}")
                nc.vector.reciprocal(
                    zrec[:, :],
                    a.rearrange("p (q w) -> p q w", w=128)[:, :, 64:65],
                )
                for qc in range(4):
                    nc.vector.tensor_scalar_mul(
                        ytp[p][qc][:, 64 * h: 64 * h + 64],
                        a[:, qc * 128: qc * 128 + 64],
                        zrec[:, qc:qc + 1],
                    )

            ytv = yt_all.rearrange("p (c t) -> p c t", t=512)

            def ytr_unit(s, qc, ccg, eng):
                p = s % 2
                b = atr if (qc * 2 + ccg) % 2 == 0 else dtr
                for j in range(4):
                    cc = ccg * 4 + j
                    nc.tensor.transpose(
                        b[:, j * 128:(j + 1) * 128],
                        ytp[p][qc][:, cc * 128:(cc + 1) * 128], ident[:],
                    )
                eng(
                    ytv[:, ccg * 4:(ccg + 1) * 4, qc * 128:(qc + 1) * 128],
                    b.rearrange("p (c t) -> p c t", t=128),
                )

            def proj_unit(s, m, vn, b, ob, eng):
                for cc in range(8):
                    nc.tensor.matmul(
                        b[:, :],
                        yt_all[:, cc * 512 + m * 128: cc * 512 + (m + 1) * 128],
                        wp_sb[cc][:, vn * 512:(vn + 1) * 512],
                        start=(cc == 0), stop=(cc == 7),
                    )
                eng(ob[:, vn * 512:(vn + 1) * 512], b[:, :])
                if vn == 1:
                    nc.sync.dma_start(
                        out[s * SEG + m * 128: s * SEG + (m + 1) * 128, :], ob[:]
                    )

            def d_units(s, pbanks, eng):
                # interleaved Y^T transpose + projection units for segment s
                units = []
                obs = {}

                def mk_ob(m):
                    obs[m] = work.tile([128, C], F32, tag="ob", bufs=2,
                                       name=f"ob{s}_{m}")
                    return obs[m]

                for qc in range(4):
                    units.append((COST_A, lambda s=s, qc=qc: ytr_unit(s, qc, 0, eng)))
                    units.append((COST_A, lambda s=s, qc=qc: ytr_unit(s, qc, 1, eng)))
                    if qc >= 1:
                        m = qc - 1
                        units.append((COST_D, lambda s=s, m=m: proj_unit(
                            s, m, 0, pbanks[m % len(pbanks)], mk_ob(m), eng)))
                        units.append((COST_D, lambda s=s, m=m: proj_unit(
                            s, m, 1, pbanks[(m + 1) % len(pbanks)], obs[m], eng)))
                units.append((COST_D, lambda s=s: proj_unit(
                    s, 3, 0, pbanks[3 % len(pbanks)], mk_ob(3), eng)))
                units.append((COST_D, lambda s=s: proj_unit(
                    s, 3, 1, pbanks[0 % len(pbanks)], obs[3], eng)))
                return units

            # ---------- program ----------
            # prologue: DMA order matters (one effective DMA pipe):
            # x(seg0) -> Wq -> Wk -> Wv -> x(seg1) -> Wproj
            dma_xn(0)
            for g in range(3):
                for cc in range(8):
                    nc.sync.dma_start(
                        wa_sb[cc][:, g * C:(g + 1) * C],
                        wa[cc * 128:(cc + 1) * 128, g * C:(g + 1) * C],
                    )
                if g == 0:
                    for cc in range(8):
                        a_unit(0, cc)
            dma_xn(1)
            for cc in range(8):
                nc.sync.dma_start(wp_sb[cc][:], wp[cc * 128:(cc + 1) * 128, :])
            # B_0 with A_1 units woven in (keeps the x^T staging drains
            # of segment 1 off the critical path of C_0's early slots)
            for u in range(16):
                qk_unit(0, u, [bu0, av[0]][u % 2])
                if u < 8:
                    a_unit(1, u)
            for u in range(8):
                v_unit(0, u, [bu0, av[0]][u % 2])

            for s in range(NSEG):
                if s + 2 < NSEG:
                    dma_xn(s + 2)
                if s + 1 < NSEG:
                    # B units of s+1; their x^T (A units of s+1) ran during
                    # the previous segment's drain region
                    fillers = [(COST_B, lambda b, s=s, u=u: qk_unit(s + 1, u, b))
                               for u in range(16)]
                    fillers += [(COST_B, lambda b, s=s, u=u: v_unit(s + 1, u, b))
                                for u in range(8)]
                else:
                    fillers = [(c, lambda b, f=f: f())
                               for c, f in d_units(s - 1, [bu0], nc.vector.tensor_copy)]
                for h in range(H):
                    head_s(s, h)
                    if h > 0:
                        head_av(s, h - 1)
                    got = big = 0
                    while fillers and got < SLOT_FILL and big < 1:
                        c, f = fillers.pop(0)
                        f(bu0)
                        got += c
                        big += c > 500
                if fillers:
                    _, f = fillers.pop(0)
                    f(bu0)
                head_av(s, H - 1)
                # drain region: leftover B units of s+1 (alternating banks)
                # interleaved with A units of s+2
                a_next = [lambda s=s, cc=cc: a_unit(s + 2, cc)
                          for cc in range(8)] if s + 2 < NSEG else []
                k = 0
                while fillers or a_next:
                    if fillers:
                        _, f = fillers.pop(0)
                        f([bu0, av[0]][k % 2])
                        k += 1
                    if a_next:
                        a_next.pop(0)()
                if s + 1 < NSEG:
                    if s == NSEG - 2:
                        pass  # D of this segment becomes next C's filler
                    else:
                        for _, f in d_units(s, [sbank(2), sbank(3)], nc.scalar.copy):
                            f()
                else:
                    for _, f in d_units(s, [sbank(2), sbank(3)], nc.scalar.copy):
                        f()

    _split_multi_waits(nc)
    return nc


_NC = None


def kernel(x, w_attn, w_proj, split_sections):
    global _NC
    if _NC is None:
        _NC = _build()
    x = np.asarray(x, dtype=np.float32).astype(ml_dtypes.bfloat16)
    w_attn = np.asarray(w_attn, dtype=np.float32).astype(ml_dtypes.bfloat16)
    w_proj = np.asarray(w_proj, dtype=np.float32).astype(ml_dtypes.bfloat16)
    in_maps = [
        {"x_sh": np.ascontiguousarray(x[i * TOK:(i + 1) * TOK]),
         "w_attn": w_attn, "w_proj": w_proj}
        for i in range(NCORES)
    ]
    res = run_bass_kernel_spmd(_NC, in_maps, core_ids=list(range(NCORES)))
    return np.concatenate([res.results[i]["out"] for i in range(NCORES)], axis=0)


if __name__ == "__main__":
    rng = np.random.default_rng(0)
    x = rng.standard_normal((T, C), dtype=np.float32)
    wa = (rng.standard_normal((C, 3 * C), dtype=np.float32) / np.sqrt(C)).astype(np.float32)
    wpj = (rng.standard_normal((C, C), dtype=np.float32) / np.sqrt(C)).astype(np.float32)
    y = kernel(x, wa, wpj, np.arange(1, 32) * 512)
    print("out", y.shape, y.dtype, np.abs(y).mean())


# revision 24
# speedup vs baseline: 1.0192x; 1.0192x over previous
"""MHSA over 32 independent 512-token segments, segment-parallel across 8
NeuronCores (4 segments / 2048 tokens per core, zero cross-core traffic).

All operands bf16 (converted host-side); matmul accumulation f32 in PSUM.
w_attn / w_proj are SBUF-resident for the whole kernel (loaded once).

Per core, per segment s:
  x^T        via PE transpose of bf16 x                   8x [128, 512]
  Q^T,K^T    = (W^T x^T) : lhsT=W chunks, rhs=x^T         16x [128, 512]
  V          = x @ Wv    : lhsT=x^T chunks, rhs=W         natural [tok, 1024]
  S^T        = K Q^T per head (K=64)                      [512k, 512q] psum
  A^T        = exp(S^T/8)  bf16 (no max-sub: |S/8|<~6)    [512, 512]
  O          = A^T.T @ [V|1] per q-chunk: out [128q, 65]  (col 64 = rowsum Z)
  Y          = O[:, 0:64] * (1/Z)  per-partition scalar   (DVE, no PE)
  Y^T        via PE transpose (bf16)
  out        = Y^T.T @ Wproj

Schedule: phase C (attention) of segment s is interleaved with x-transpose /
QKV matmuls of segment s+1 (and, for the last segment, with the previous
segment's projection) so the PE never waits on the Act-bound exp chain.

PSUM banks: 0-3 S^T staging (head-parity pairs), 4-5 A.V outputs,
6 QKV-unit accumulation, 7 (bf16) transpose staging halves.
"""

import numpy as np
import ml_dtypes

import concourse.bass as bass
import concourse.mybir as mybir
import concourse.tile as tile
from concourse.bass_utils import run_bass_kernel_spmd

F32 = mybir.dt.float32
BF16 = mybir.dt.bfloat16
EXP = mybir.ActivationFunctionType.Exp

T, C, H, HD = 16384, 1024, 16, 64
NCORES = 8
TOK = T // NCORES          # 2048 tokens per core
SEG = 512                  # tokens per segment
NSEG = TOK // SEG          # 4 segments per core
SCALE = 1.0 / np.sqrt(HD)  # folded into exp()

# filler unit PE costs (ns) for pacing phase-C interleave
COST_A, COST_B, COST_D = 220, 1710, 1750
SLOT_FILL = 1050           # target filler ns per head slot


def _split_multi_waits(nc):
    """Drop provably-satisfied same-engine waits, then move any remaining
    extra sync waits onto same-engine NoOps (1-wait ISA limit).

    A wait on the instruction's own engine's semaphore with wait_value <=
    (count of prior same-engine updates to that semaphore) is guaranteed by
    in-order engine execution; keeping it (or worse, splitting it onto a
    NoOp) stalls the sequencer until the previous instruction completes and
    the semaphore propagates, serializing back-to-back engine work.
    """
    for fn in nc.m.functions:
        for bb in fn.blocks:
            out = []
            for inst in bb.instructions:
                si = inst.sync_info
                eng = str(inst.engine).split(".")[-1]
                if si is not None and si.on_wait and len(si.on_wait) > 1:
                    # same-engine waits go on NoOps (deep buffer rotation
                    # makes them reference old, already-fired semaphores);
                    # the cross-engine wait stays on the instruction so the
                    # sequencer can queue ahead while it pends
                    waits = sorted(
                        si.on_wait,
                        key=lambda w: not (
                            isinstance(w.ant_name, str)
                            and w.ant_name.split("_")[0] == eng
                        ),
                    )
                    for j, w in enumerate(waits[:-1]):
                        nop = mybir.InstNoOp(name=f"{inst.name}-wsp{j}")
                        nop.engine = inst.engine
                        nop.sync_info = mybir.SyncInfo(on_wait=[w], on_update=[])
                        out.append(nop)
                    inst.sync_info = mybir.SyncInfo(
                        on_wait=[waits[-1]], on_update=list(si.on_update)
                    )
                out.append(inst)
            bb.instructions = out


def _build():
    nc = bass.Bass("TRN2", target_bir_lowering=False, debug=False)
    x = nc.dram_tensor("x_sh", [TOK, C], BF16, kind="ExternalInput").ap()
    wa = nc.dram_tensor("w_attn", [C, 3 * C], BF16, kind="ExternalInput").ap()
    wp = nc.dram_tensor("w_proj", [C, C], BF16, kind="ExternalInput").ap()
    out = nc.dram_tensor("out", [TOK, C], F32, kind="ExternalOutput").ap()

    ident_d = nc.inline_tensor(
        np.eye(128, dtype=np.float32).astype(ml_dtypes.bfloat16), "ident_c"
    ).ap()

    with tile.TileContext(nc) as tc:
        with (
            tc.tile_pool(name="const", bufs=1) as cpool,
            tc.tile_pool(name="wres", bufs=1) as wres,
            tc.tile_pool(name="stream", bufs=1) as stream,
            tc.tile_pool(name="work", bufs=1) as work,
            tc.tile_pool(name="ps", bufs=1, space="PSUM") as pspool,
        ):
            # ---- PSUM layout (allocated in order, bank = 512 f32)
            psA = pspool.tile([128, 2048], F32, tag="psA", name="psA")   # banks 0-3
            av = [pspool.tile([128, 512], F32, tag=f"av{i}", name=f"av{i}")
                  for i in range(2)]                                     # banks 4-5
            bu0 = pspool.tile([128, 512], F32, tag="bu0", name="bu0")    # bank 6
            trb = pspool.tile([128, 1024], BF16, tag="trb", name="trb")  # bank 7
            atr = trb[:, 0:512]
            dtr = trb[:, 512:1024]

            def sbank(i):
                return psA[:, 512 * i:512 * (i + 1)]

            ident = cpool.tile([128, 128], BF16, tag="ident", name="ident")
            nc.sync.dma_start(ident[:], ident_d[:, :])

            # ---- resident weights (bf16, loaded once), wa split per QKV group
            wa_sb = [wres.tile([128, 3 * C], BF16, tag=f"wa{cc}", name=f"wa{cc}")
                     for cc in range(8)]
            wp_sb = [wres.tile([128, C], BF16, tag=f"wp{cc}", name=f"wp{cc}")
                     for cc in range(8)]

            # ---- per-segment working tiles, double-buffered by parity
            xn = [[stream.tile([128, C], BF16, tag=f"xn{p}_{qt}", name=f"xn{p}_{qt}")
                   for qt in range(4)] for p in range(2)]
            xT = [[work.tile([128, SEG], BF16, tag=f"xT{p}_{cc}", name=f"xT{p}_{cc}")
                   for cc in range(8)] for p in range(2)]
            qkt = [[work.tile([128, SEG], BF16, tag=f"qkt{p}_{m}", name=f"qkt{p}_{m}")
                    for m in range(16)] for p in range(2)]
            vp = [[work.tile([128, 16 * 66], BF16, tag=f"vp{p}_{qt}", name=f"vp{p}_{qt}")
                   for qt in range(4)] for p in range(2)]
            ytp = [[work.tile([128, C], BF16, tag=f"ytp{p}_{qc}", name=f"ytp{p}_{qc}")
                    for qc in range(4)] for p in range(2)]
            yt_all = work.tile([128, 4096], BF16, tag="yt", name="yt")

            # ones columns of vp (col 64 of each 66-stride head block) persist
            for p in range(2):
                for qt in range(4):
                    nc.vector.memset(
                        vp[p][qt].rearrange("p (h w) -> p h w", w=66)[:, :, 64:65], 1.0
                    )

            # ---------- emission helpers ----------
            def dma_xn(s):
                p = s % 2
                for qt in range(4):
                    nc.sync.dma_start(
                        xn[p][qt][:],
                        x[s * SEG + qt * 128: s * SEG + (qt + 1) * 128, :],
                    )

            def a_unit(s, cc):
                # x^T for channel chunk cc: 4 transposes into bf16 staging
                p = s % 2
                st = atr if cc % 2 == 0 else dtr
                for qt in range(4):
                    nc.tensor.transpose(
                        st[:, qt * 128:(qt + 1) * 128],
                        xn[p][qt][:, cc * 128:(cc + 1) * 128], ident[:],
                    )
                nc.vector.tensor_copy(xT[p][cc][:], st[:, :])

            def qk_unit(s, u, b):
                # Q^T/K^T channel chunk: g = u//8 (0=Q,1=K), m = u%8
                p = s % 2
                g, m = u // 8, u % 8
                for cc in range(8):
                    nc.tensor.matmul(
                        b[:, :],
                        wa_sb[cc][:, g * C + m * 128: g * C + (m + 1) * 128],
                        xT[p][cc][:],
                        start=(cc == 0), stop=(cc == 7),
                    )
                nc.vector.tensor_copy(qkt[p][g * 8 + m][:], b[:, :])

            def v_unit(s, u, b):
                # V tok-chunk qt = u//2, channel half vn = u%2
                p = s % 2
                qt, vn = u // 2, u % 2
                for cc in range(8):
                    nc.tensor.matmul(
                        b[:, :],
                        xT[p][cc][:, qt * 128:(qt + 1) * 128],
                        wa_sb[cc][:, 2 * C + vn * 512: 2 * C + (vn + 1) * 512],
                        start=(cc == 0), stop=(cc == 7),
                    )
                nc.vector.tensor_copy(
                    vp[p][qt].rearrange("p (h w) -> p h w", w=66)[:, vn * 8:(vn + 1) * 8, 0:64],
                    b.rearrange("p (h w) -> p h w", w=64),
                )

            at0s = {}

            def head_s(s, h):
                # S^T chunks into banks 0-3 + two exp halves -> at0 (bf16)
                p = s % 2
                qk_q = qkt[p][h // 2]
                qk_k = qkt[p][8 + h // 2]
                r0 = (h % 2) * 64
                at0 = at0s[h % 3] = work.tile([128, 2048], BF16, tag="at0",
                                              bufs=3, name=f"at0_{s}_{h}")
                for j in range(2):
                    for i in range(2):
                        kt = j * 2 + i
                        nc.tensor.matmul(
                            sbank(kt),
                            qk_k[r0:r0 + 64, kt * 128:(kt + 1) * 128],
                            qk_q[r0:r0 + 64, :], start=True, stop=True,
                        )
                    nc.scalar.activation(
                        at0[:, j * 1024:(j + 1) * 1024],
                        psA[:, j * 1024:(j + 1) * 1024], EXP, scale=SCALE,
                    )

            def head_av(s, h):
                p = s % 2
                at0 = at0s[h % 3]
                a = av[h % 2]
                for qc in range(4):
                    for kt in range(4):
                        nc.tensor.matmul(
                            a[:, qc * 128: qc * 128 + 65],
                            at0[:, kt * 512 + qc * 128: kt * 512 + (qc + 1) * 128],
                            vp[p][kt][:, 66 * h: 66 * h + 65],
                            start=(kt == 0), stop=(kt == 3),
                        )
                zrec = work.tile([128, 4], F32, tag="zrec", bufs=4,
                                 name=f"zrec_{s}_{h}")
                nc.vector.reciprocal(
                    zrec[:, :],
                    a.rearrange("p (q w) -> p q w", w=128)[:, :, 64:65],
                )
                for qc in range(4):
                    nc.vector.tensor_scalar_mul(
                        ytp[p][qc][:, 64 * h: 64 * h + 64],
                        a[:, qc * 128: qc * 128 + 64],
                        zrec[:, qc:qc + 1],
                    )

            ytv = yt_all.rearrange("p (c t) -> p c t", t=512)

            def ytr_unit(s, qc, ccg, eng):
                p = s % 2
                b = atr if (qc * 2 + ccg) % 2 == 0 else dtr
                for j in range(4):
                    cc = ccg * 4 + j
                    nc.tensor.transpose(
                        b[:, j * 128:(j + 1) * 128],
                        ytp[p][qc][:, cc * 128:(cc + 1) * 128], ident[:],
                    )
                eng(
                    ytv[:, ccg * 4:(ccg + 1) * 4, qc * 128:(qc + 1) * 128],
                    b.rearrange("p (c t) -> p c t", t=128),
                )

            def proj_unit(s, m, vn, b, ob, eng):
                for cc in range(8):
                    nc.tensor.matmul(
                        b[:, :],
                        yt_all[:, cc * 512 + m * 128: cc * 512 + (m + 1) * 128],
                        wp_sb[cc][:, vn * 512:(vn + 1) * 512],
                        start=(cc == 0), stop=(cc == 7),
                    )
                eng(ob[:, vn * 512:(vn + 1) * 512], b[:, :])
                if vn == 1:
                    nc.sync.dma_start(
                        out[s * SEG + m * 128: s * SEG + (m + 1) * 128, :], ob[:]
                    )

            def d_units(s, pbanks, eng):
                # interleaved Y^T transpose + projection units for segment s
                units = []
                obs = {}

                def mk_ob(m):
                    obs[m] = work.tile([128, C], F32, tag="ob", bufs=2,
                                       name=f"ob{s}_{m}")
                    return obs[m]

                for qc in range(4):
                    units.append((COST_A, lambda s=s, qc=qc: ytr_unit(s, qc, 0, eng)))
                    units.append((COST_A, lambda s=s, qc=qc: ytr_unit(s, qc, 1, eng)))
                    if qc >= 1:
                        m = qc - 1
                        units.append((COST_D, lambda s=s, m=m: proj_unit(
                            s, m, 0, pbanks[m % len(pbanks)], mk_ob(m), eng)))
                        units.append((COST_D, lambda s=s, m=m: proj_unit(
                            s, m, 1, pbanks[(m + 1) % len(pbanks)], obs[m], eng)))
                units.append((COST_D, lambda s=s: proj_unit(
                    s, 3, 0, pbanks[3 % len(pbanks)], mk_ob(3), eng)))
                units.append((COST_D, lambda s=s: proj_unit(
                    s, 3, 1, pbanks[0 % len(pbanks)], obs[3], eng)))
                return units

            # ---------- program ----------
            # prologue: DMA order matters (one effective DMA pipe):
            # x(seg0) -> Wq -> Wk -> Wv -> x(seg1) -> Wproj
            dma_xn(0)
            for g in range(3):
                for cc in range(8):
                    nc.sync.dma_start(
                        wa_sb[cc][:, g * C:(g + 1) * C],
                        wa[cc * 128:(cc + 1) * 128, g * C:(g + 1) * C],
                    )
                if g == 0:
                    for cc in range(8):
                        a_unit(0, cc)
            dma_xn(1)
            for cc in range(8):
                nc.sync.dma_start(wp_sb[cc][:], wp[cc * 128:(cc + 1) * 128, :])
            # B_0 with A_1 units woven in (keeps the x^T staging drains
            # of segment 1 off the critical path of C_0's early slots)
            for u in range(16):
                qk_unit(0, u, [bu0, av[0]][u % 2])
                if u < 8:
                    a_unit(1, u)
            for u in range(8):
                v_unit(0, u, [bu0, av[0]][u % 2])

            for s in range(NSEG):
                if s + 2 < NSEG:
                    dma_xn(s + 2)
                if s + 1 < NSEG:
                    # B units of s+1; their x^T (A units of s+1) ran during
                    # the previous segment's drain region
                    fillers = [(COST_B, lambda b, s=s, u=u: qk_unit(s + 1, u, b))
                               for u in range(16)]
                    fillers += [(COST_B, lambda b, s=s, u=u: v_unit(s + 1, u, b))
                                for u in range(8)]
                else:
                    fillers = [(c, lambda b, f=f: f())
                               for c, f in d_units(s - 1, [bu0], nc.vector.tensor_copy)]
                for h in range(H):
                    head_s(s, h)
                    if h > 0:
                        head_av(s, h - 1)
                    got = big = 0
                    while fillers and got < SLOT_FILL and big < 1:
                        c, f = fillers.pop(0)
                        f(bu0)
                        got += c
                        big += c > 500
                if fillers:
                    _, f = fillers.pop(0)
                    f(bu0)
                head_av(s, H - 1)
                # drain region: leftover B units of s+1 (alternating banks)
                # interleaved with A units of s+2
                a_next = [lambda s=s, cc=cc: a_unit(s + 2, cc)
                          for cc in range(8)] if s + 2 < NSEG else []
                k = 0
                while fillers or a_next:
                    if fillers:
                        _, f = fillers.pop(0)
                        f([bu0, av[0]][k % 2])
                        k += 1
                    if a_next:
                        a_next.pop(0)()
                if s + 1 < NSEG:
                    if s == NSEG - 2:
                        pass  # D of this segment becomes next C's filler
                    else:
                        for _, f in d_units(s, [sbank(2), sbank(3)], nc.scalar.copy):
                            f()
                else:
                    for _, f in d_units(s, [sbank(2), sbank(3)], nc.scalar.copy):
                        f()

    _split_multi_waits(nc)
    return nc


_NC = None


def kernel(x, w_attn, w_proj, split_sections):
    global _NC
    if _NC is None:
        _NC = _build()
    x = np.asarray(x, dtype=np.float32).astype(ml_dtypes.bfloat16)
    w_attn = np.asarray(w_attn, dtype=np.float32).astype(ml_dtypes.bfloat16)
    w_proj = np.asarray(w_proj, dtype=np.float32).astype(ml_dtypes.bfloat16)
    in_maps = [
        {"x_sh": np.ascontiguousarray(x[i * TOK:(i + 1) * TOK]),
         "w_attn": w_attn, "w_proj": w_proj}
        for i in range(NCORES)
    ]
    res = run_bass_kernel_spmd(_NC, in_maps, core_ids=list(range(NCORES)))
    return np.concatenate([res.results[i]["out"] for i in range(NCORES)], axis=0)


if __name__ == "__main__":
    rng = np.random.default_rng(0)
    x = rng.standard_normal((T, C), dtype=np.float32)
    wa = (rng.standard_normal((C, 3 * C), dtype=np.float32) / np.sqrt(C)).astype(np.float32)
    wpj = (rng.standard_normal((C, C), dtype=np.float32) / np.sqrt(C)).astype(np.float32)
    y = kernel(x, wa, wpj, np.arange(1, 32) * 512)
    print("out", y.shape, y.dtype, np.abs(y).mean())


# revision 27
# speedup vs baseline: 1.1546x; 1.1328x over previous
"""MHSA over 32 independent 512-token segments, segment-parallel across 8
NeuronCores (4 segments / 2048 tokens per core, zero cross-core traffic).

All operands bf16 (converted host-side); matmul accumulation f32 in PSUM.
w_attn / w_proj are SBUF-resident for the whole kernel (loaded once).

Per core, per segment s:
  x^T        via PE transpose of bf16 x                   8x [128, 512]
  Q^T,K^T    = (W^T x^T) : lhsT=W chunks, rhs=x^T         16x [128, 512]
  V          = x @ Wv    : lhsT=x^T chunks, rhs=W         natural [tok, 1024]
  S^T        = K Q^T per head (K=64)                      [512k, 512q] psum
  A^T        = exp(S^T/8)  bf16 (no max-sub: |S/8|<~6)    [512, 512]
  O          = A^T.T @ [V|1] per q-chunk: out [128q, 65]  (col 64 = rowsum Z)
  Y          = O[:, 0:64] * (1/Z)  per-partition scalar   (DVE, no PE)
  Y^T        via PE transpose (bf16)
  out        = Y^T.T @ Wproj

Schedule: phase C (attention) of segment s is interleaved with x-transpose /
QKV matmuls of segment s+1 (and, for the last segment, with the previous
segment's projection) so the PE never waits on the Act-bound exp chain.

PSUM banks: 0-3 S^T staging (head-parity pairs), 4-5 A.V outputs,
6 QKV-unit accumulation, 7 (bf16) transpose staging halves.
"""

import numpy as np
import ml_dtypes

import concourse.bass as bass
import concourse.mybir as mybir
import concourse.tile as tile
from concourse.bass_utils import run_bass_kernel_spmd

F32 = mybir.dt.float32
BF16 = mybir.dt.bfloat16
EXP = mybir.ActivationFunctionType.Exp

T, C, H, HD = 16384, 1024, 16, 64
NCORES = 8
TOK = T // NCORES          # 2048 tokens per core
SEG = 512                  # tokens per segment
NSEG = TOK // SEG          # 4 segments per core
SCALE = 1.0 / np.sqrt(HD)  # folded into exp()

# filler unit PE costs (ns) for pacing phase-C interleave
COST_A, COST_B, COST_D = 220, 1710, 1750
SLOT_FILL = 1050           # target filler ns per head slot


def _split_multi_waits(nc):
    """Drop provably-satisfied same-engine waits, then move any remaining
    extra sync waits onto same-engine NoOps (1-wait ISA limit).

    A wait on the instruction's own engine's semaphore with wait_value <=
    (count of prior same-engine updates to that semaphore) is guaranteed by
    in-order engine execution; keeping it (or worse, splitting it onto a
    NoOp) stalls the sequencer until the previous instruction completes and
    the semaphore propagates, serializing back-to-back engine work.
    """
    for fn in nc.m.functions:
        for bb in fn.blocks:
            out = []
            for inst in bb.instructions:
                si = inst.sync_info
                eng = str(inst.engine).split(".")[-1]
                if si is not None and si.on_wait and len(si.on_wait) > 1:
                    # same-engine waits go on NoOps (deep buffer rotation
                    # makes them reference old, already-fired semaphores);
                    # the cross-engine wait stays on the instruction so the
                    # sequencer can queue ahead while it pends
                    waits = sorted(
                        si.on_wait,
                        key=lambda w: not (
                            isinstance(w.ant_name, str)
                            and w.ant_name.split("_")[0] == eng
                        ),
                    )
                    for j, w in enumerate(waits[:-1]):
                        nop = mybir.InstNoOp(name=f"{inst.name}-wsp{j}")
                        nop.engine = inst.engine
                        nop.sync_info = mybir.SyncInfo(on_wait=[w], on_update=[])
                        out.append(nop)
                    inst.sync_info = mybir.SyncInfo(
                        on_wait=[waits[-1]], on_update=list(si.on_update)
                    )
                out.append(inst)
            bb.instructions = out


def _build():
    nc = bass.Bass("TRN2", target_bir_lowering=False, debug=False)
    x = nc.dram_tensor("x_sh", [TOK, C], BF16, kind="ExternalInput").ap()
    wa = nc.dram_tensor("w_attn", [C, 3 * C], BF16, kind="ExternalInput").ap()
    wp = nc.dram_tensor("w_proj", [C, C], BF16, kind="ExternalInput").ap()
    out = nc.dram_tensor("out", [TOK, C], F32, kind="ExternalOutput").ap()

    ident_d = nc.inline_tensor(
        np.eye(128, dtype=np.float32).astype(ml_dtypes.bfloat16), "ident_c"
    ).ap()

    with tile.TileContext(nc) as tc:
        with (
            tc.tile_pool(name="const", bufs=1) as cpool,
            tc.tile_pool(name="wres", bufs=1) as wres,
            tc.tile_pool(name="stream", bufs=1) as stream,
            tc.tile_pool(name="work", bufs=1) as work,
            tc.tile_pool(name="ps", bufs=1, space="PSUM") as pspool,
        ):
            # ---- PSUM layout: separate tiles (sync tracking is per-tile,
            # a shared tile would serialize bank rotation at tile granularity)
            sb = [pspool.tile([128, 512], F32, tag=f"sb{i}", name=f"sb{i}")
                  for i in range(3)]                                     # banks 0-2
            av = [pspool.tile([128, 512], F32, tag=f"av{i}", name=f"av{i}")
                  for i in range(2)]                                     # banks 3-4
            bu0 = pspool.tile([128, 512], F32, tag="bu0", name="bu0")    # bank 5
            atr = pspool.tile([128, 512], BF16, tag="atr", name="atr")   # bank 6
            dtr = pspool.tile([128, 512], BF16, tag="dtr", name="dtr")   # bank 7

            def sbank(i):
                return sb[i][:, :]

            ident = cpool.tile([128, 128], BF16, tag="ident", name="ident")
            nc.sync.dma_start(ident[:], ident_d[:, :])

            # ---- resident weights (bf16, loaded once), wa split per QKV group
            wa_sb = [wres.tile([128, 3 * C], BF16, tag=f"wa{cc}", name=f"wa{cc}")
                     for cc in range(8)]
            wp_sb = [wres.tile([128, C], BF16, tag=f"wp{cc}", name=f"wp{cc}")
                     for cc in range(8)]

            # ---- per-segment working tiles, double-buffered by parity
            xn = [[stream.tile([128, C], BF16, tag=f"xn{p}_{qt}", name=f"xn{p}_{qt}")
                   for qt in range(4)] for p in range(2)]
            xT = [[work.tile([128, SEG], BF16, tag=f"xT{p}_{cc}", name=f"xT{p}_{cc}")
                   for cc in range(8)] for p in range(2)]
            qkt = [[work.tile([128, SEG], BF16, tag=f"qkt{p}_{m}", name=f"qkt{p}_{m}")
                    for m in range(16)] for p in range(2)]
            vp = [[work.tile([128, 16 * 66], BF16, tag=f"vp{p}_{qt}", name=f"vp{p}_{qt}")
                   for qt in range(4)] for p in range(2)]
            ytp = [[work.tile([128, C], BF16, tag=f"ytp{p}_{qc}", name=f"ytp{p}_{qc}")
                    for qc in range(4)] for p in range(2)]
            yt_all = work.tile([128, 4096], BF16, tag="yt", name="yt")

            # ones columns of vp (col 64 of each 66-stride head block) persist
            for p in range(2):
                for qt in range(4):
                    nc.vector.memset(
                        vp[p][qt].rearrange("p (h w) -> p h w", w=66)[:, :, 64:65], 1.0
                    )

            # ---------- emission helpers ----------
            def dma_xn(s):
                p = s % 2
                for qt in range(4):
                    nc.sync.dma_start(
                        xn[p][qt][:],
                        x[s * SEG + qt * 128: s * SEG + (qt + 1) * 128, :],
                    )

            def a_unit(s, cc):
                # x^T for channel chunk cc: 4 transposes into bf16 staging
                p = s % 2
                st = atr if cc % 2 == 0 else dtr
                for qt in range(4):
                    nc.tensor.transpose(
                        st[:, qt * 128:(qt + 1) * 128],
                        xn[p][qt][:, cc * 128:(cc + 1) * 128], ident[:],
                    )
                nc.vector.tensor_copy(xT[p][cc][:], st[:, :])

            def qk_unit(s, u, b):
                # Q^T/K^T channel chunk: g = u//8 (0=Q,1=K), m = u%8
                p = s % 2
                g, m = u // 8, u % 8
                for cc in range(8):
                    nc.tensor.matmul(
                        b[:, :],
                        wa_sb[cc][:, g * C + m * 128: g * C + (m + 1) * 128],
                        xT[p][cc][:],
                        start=(cc == 0), stop=(cc == 7),
                    )
                nc.vector.tensor_copy(qkt[p][g * 8 + m][:], b[:, :])

            def v_unit(s, u, b):
                # V tok-chunk qt = u//2, channel half vn = u%2
                p = s % 2
                qt, vn = u // 2, u % 2
                for cc in range(8):
                    nc.tensor.matmul(
                        b[:, :],
                        xT[p][cc][:, qt * 128:(qt + 1) * 128],
                        wa_sb[cc][:, 2 * C + vn * 512: 2 * C + (vn + 1) * 512],
                        start=(cc == 0), stop=(cc == 7),
                    )
                nc.vector.tensor_copy(
                    vp[p][qt].rearrange("p (h w) -> p h w", w=66)[:, vn * 8:(vn + 1) * 8, 0:64],
                    b.rearrange("p (h w) -> p h w", w=64),
                )

            at0s = {}

            def head_s(s, h):
                # S^T chunks into banks 0-3 + two exp halves -> at0 (bf16)
                p = s % 2
                qk_q = qkt[p][h // 2]
                qk_k = qkt[p][8 + h // 2]
                r0 = (h % 2) * 64
                at0 = at0s[h % 3] = work.tile([128, 2048], BF16, tag="at0",
                                              bufs=3, name=f"at0_{s}_{h}")
                for kt in range(4):
                    b = sbank((h * 4 + kt) % 3)
                    nc.tensor.matmul(
                        b, qk_k[r0:r0 + 64, kt * 128:(kt + 1) * 128],
                        qk_q[r0:r0 + 64, :], start=True, stop=True,
                    )
                    nc.scalar.activation(
                        at0[:, kt * 512:(kt + 1) * 512], b, EXP, scale=SCALE,
                    )

            def head_av(s, h):
                p = s % 2
                at0 = at0s[h % 3]
                a = av[h % 2]
                for qc in range(4):
                    for kt in range(4):
                        nc.tensor.matmul(
                            a[:, qc * 128: qc * 128 + 65],
                            at0[:, kt * 512 + qc * 128: kt * 512 + (qc + 1) * 128],
                            vp[p][kt][:, 66 * h: 66 * h + 65],
                            start=(kt == 0), stop=(kt == 3),
                        )
                zrec = work.tile([128, 4], F32, tag="zrec", bufs=4,
                                 name=f"zrec_{s}_{h}")
                nc.vector.reciprocal(
                    zrec[:, :],
                    a.rearrange("p (q w) -> p q w", w=128)[:, :, 64:65],
                )
                for qc in range(4):
                    nc.vector.tensor_scalar_mul(
                        ytp[p][qc][:, 64 * h: 64 * h + 64],
                        a[:, qc * 128: qc * 128 + 64],
                        zrec[:, qc:qc + 1],
                    )

            ytv = yt_all.rearrange("p (c t) -> p c t", t=512)

            def ytr_unit(s, qc, ccg, eng):
                p = s % 2
                b = atr if (qc * 2 + ccg) % 2 == 0 else dtr
                for j in range(4):
                    cc = ccg * 4 + j
                    nc.tensor.transpose(
                        b[:, j * 128:(j + 1) * 128],
                        ytp[p][qc][:, cc * 128:(cc + 1) * 128], ident[:],
                    )
                eng(
                    ytv[:, ccg * 4:(ccg + 1) * 4, qc * 128:(qc + 1) * 128],
                    b.rearrange("p (c t) -> p c t", t=128),
                )

            def proj_unit(s, m, vn, b, ob, eng):
                for cc in range(8):
                    nc.tensor.matmul(
                        b[:, :],
                        yt_all[:, cc * 512 + m * 128: cc * 512 + (m + 1) * 128],
                        wp_sb[cc][:, vn * 512:(vn + 1) * 512],
                        start=(cc == 0), stop=(cc == 7),
                    )
                eng(ob[:, vn * 512:(vn + 1) * 512], b[:, :])
                if vn == 1:
                    nc.sync.dma_start(
                        out[s * SEG + m * 128: s * SEG + (m + 1) * 128, :], ob[:]
                    )

            def d_units(s, pbanks, eng):
                # interleaved Y^T transpose + projection units for segment s
                units = []
                obs = {}

                def mk_ob(m):
                    obs[m] = work.tile([128, C], F32, tag="ob", bufs=2,
                                       name=f"ob{s}_{m}")
                    return obs[m]

                for qc in range(4):
                    units.append((COST_A, lambda s=s, qc=qc: ytr_unit(s, qc, 0, eng)))
                    units.append((COST_A, lambda s=s, qc=qc: ytr_unit(s, qc, 1, eng)))
                    if qc >= 1:
                        m = qc - 1
                        units.append((COST_D, lambda s=s, m=m: proj_unit(
                            s, m, 0, pbanks[m % len(pbanks)], mk_ob(m), eng)))
                        units.append((COST_D, lambda s=s, m=m: proj_unit(
                            s, m, 1, pbanks[(m + 1) % len(pbanks)], obs[m], eng)))
                units.append((COST_D, lambda s=s: proj_unit(
                    s, 3, 0, pbanks[3 % len(pbanks)], mk_ob(3), eng)))
                units.append((COST_D, lambda s=s: proj_unit(
                    s, 3, 1, pbanks[0 % len(pbanks)], obs[3], eng)))
                return units

            # ---------- program ----------
            # prologue: DMA order matters (one effective DMA pipe):
            # x(seg0) -> Wq -> Wk -> Wv -> x(seg1) -> Wproj
            dma_xn(0)
            for g in range(3):
                for cc in range(8):
                    nc.sync.dma_start(
                        wa_sb[cc][:, g * C:(g + 1) * C],
                        wa[cc * 128:(cc + 1) * 128, g * C:(g + 1) * C],
                    )
                if g == 0:
                    for cc in range(8):
                        a_unit(0, cc)
            dma_xn(1)
            for cc in range(8):
                nc.sync.dma_start(wp_sb[cc][:], wp[cc * 128:(cc + 1) * 128, :])
            # B_0 with A_1 units woven in (keeps the x^T staging drains
            # of segment 1 off the critical path of C_0's early slots)
            for u in range(16):
                qk_unit(0, u, [bu0, av[0]][u % 2])
                if u < 8:
                    a_unit(1, u)
            for u in range(8):
                v_unit(0, u, [bu0, av[0]][u % 2])

            for s in range(NSEG):
                if s + 2 < NSEG:
                    dma_xn(s + 2)
                if s + 1 < NSEG:
                    # B units of s+1; their x^T (A units of s+1) ran during
                    # the previous segment's drain region
                    fillers = [(COST_B, lambda b, s=s, u=u: qk_unit(s + 1, u, b))
                               for u in range(16)]
                    fillers += [(COST_B, lambda b, s=s, u=u: v_unit(s + 1, u, b))
                                for u in range(8)]
                else:
                    fillers = [(c, lambda b, f=f: f())
                               for c, f in d_units(s - 1, [bu0], nc.vector.tensor_copy)]
                for h in range(H):
                    head_s(s, h)
                    if h > 0:
                        head_av(s, h - 1)
                    got = big = 0
                    while fillers and got < SLOT_FILL and big < 1:
                        c, f = fillers.pop(0)
                        f(bu0)
                        got += c
                        big += c > 500
                if fillers:
                    _, f = fillers.pop(0)
                    f(bu0)
                head_av(s, H - 1)
                # drain region: leftover B units of s+1 (alternating banks)
                # interleaved with A units of s+2
                a_next = [lambda s=s, cc=cc: a_unit(s + 2, cc)
                          for cc in range(8)] if s + 2 < NSEG else []
                k = 0
                while fillers or a_next:
                    if fillers:
                        _, f = fillers.pop(0)
                        f([bu0, av[0]][k % 2])
                        k += 1
                    if a_next:
                        a_next.pop(0)()
                if s + 1 < NSEG:
                    if s == NSEG - 2:
                        pass  # D of this segment becomes next C's filler
                    else:
                        for _, f in d_units(s, [sbank(1), sbank(2)], nc.scalar.copy):
                            f()
                else:
                    for _, f in d_units(s, [sbank(1), sbank(2)], nc.scalar.copy):
                        f()

    _split_multi_waits(nc)
    return nc


_NC = None


def kernel(x, w_attn, w_proj, split_sections):
    global _NC
    if _NC is None:
        _NC = _build()
    x = np.asarray(x, dtype=np.float32).astype(ml_dtypes.bfloat16)
    w_attn = np.asarray(w_attn, dtype=np.float32).astype(ml_dtypes.bfloat16)
    w_proj = np.asarray(w_proj, dtype=np.float32).astype(ml_dtypes.bfloat16)
    in_maps = [
        {"x_sh": np.ascontiguousarray(x[i * TOK:(i + 1) * TOK]),
         "w_attn": w_attn, "w_proj": w_proj}
        for i in range(NCORES)
    ]
    res = run_bass_kernel_spmd(_NC, in_maps, core_ids=list(range(NCORES)))
    return np.concatenate([res.results[i]["out"] for i in range(NCORES)], axis=0)


if __name__ == "__main__":
    rng = np.random.default_rng(0)
    x = rng.standard_normal((T, C), dtype=np.float32)
    wa = (rng.standard_normal((C, 3 * C), dtype=np.float32) / np.sqrt(C)).astype(np.float32)
    wpj = (rng.standard_normal((C, C), dtype=np.float32) / np.sqrt(C)).astype(np.float32)
    y = kernel(x, wa, wpj, np.arange(1, 32) * 512)
    print("out", y.shape, y.dtype, np.abs(y).mean())


# revision 29
# speedup vs baseline: 1.1772x; 1.0196x over previous
"""MHSA over 32 independent 512-token segments, segment-parallel across 8
NeuronCores (4 segments / 2048 tokens per core, zero cross-core traffic).

All operands bf16 (converted host-side); matmul accumulation f32 in PSUM.
w_attn / w_proj are SBUF-resident for the whole kernel (loaded once).

Per core, per segment s:
  x^T        via PE transpose of bf16 x                   8x [128, 512]
  Q^T,K^T    = (W^T x^T) : lhsT=W chunks, rhs=x^T         16x [128, 512]
  V          = x @ Wv    : lhsT=x^T chunks, rhs=W         natural [tok, 1024]
  S^T        = K Q^T per head (K=64)                      [512k, 512q] psum
  A^T        = exp(S^T/8)  bf16 (no max-sub: |S/8|<~6)    [512, 512]
  O          = A^T.T @ [V|1] per q-chunk: out [128q, 65]  (col 64 = rowsum Z)
  Y          = O[:, 0:64] * (1/Z)  per-partition scalar   (DVE, no PE)
  Y^T        via PE transpose (bf16)
  out        = Y^T.T @ Wproj

Schedule: phase C (attention) of segment s is interleaved with x-transpose /
QKV matmuls of segment s+1 (and, for the last segment, with the previous
segment's projection) so the PE never waits on the Act-bound exp chain.

PSUM banks: 0-3 S^T staging (head-parity pairs), 4-5 A.V outputs,
6 QKV-unit accumulation, 7 (bf16) transpose staging halves.
"""

import numpy as np
import ml_dtypes

import concourse.bass as bass
import concourse.mybir as mybir
import concourse.tile as tile
from concourse.bass_utils import run_bass_kernel_spmd

F32 = mybir.dt.float32
BF16 = mybir.dt.bfloat16
EXP = mybir.ActivationFunctionType.Exp

T, C, H, HD = 16384, 1024, 16, 64
NCORES = 8
TOK = T // NCORES          # 2048 tokens per core
SEG = 512                  # tokens per segment
NSEG = TOK // SEG          # 4 segments per core
SCALE = 1.0 / np.sqrt(HD)  # folded into exp()

# filler unit PE costs (ns) for pacing phase-C interleave
COST_A, COST_B, COST_D = 220, 1710, 1750
SLOT_FILL = 1050           # target filler ns per head slot


def _split_multi_waits(nc):
    """Drop provably-satisfied same-engine waits, then move any remaining
    extra sync waits onto same-engine NoOps (1-wait ISA limit).

    A wait on the instruction's own engine's semaphore with wait_value <=
    (count of prior same-engine updates to that semaphore) is guaranteed by
    in-order engine execution; keeping it (or worse, splitting it onto a
    NoOp) stalls the sequencer until the previous instruction completes and
    the semaphore propagates, serializing back-to-back engine work.
    """
    for fn in nc.m.functions:
        for bb in fn.blocks:
            out = []
            for inst in bb.instructions:
                si = inst.sync_info
                eng = str(inst.engine).split(".")[-1]
                if si is not None and si.on_wait and len(si.on_wait) > 1:
                    # same-engine waits go on NoOps (deep buffer rotation
                    # makes them reference old, already-fired semaphores);
                    # the cross-engine wait stays on the instruction so the
                    # sequencer can queue ahead while it pends
                    waits = sorted(
                        si.on_wait,
                        key=lambda w: not (
                            isinstance(w.ant_name, str)
                            and w.ant_name.split("_")[0] == eng
                        ),
                    )
                    for j, w in enumerate(waits[:-1]):
                        nop = mybir.InstNoOp(name=f"{inst.name}-wsp{j}")
                        nop.engine = inst.engine
                        nop.sync_info = mybir.SyncInfo(on_wait=[w], on_update=[])
                        out.append(nop)
                    inst.sync_info = mybir.SyncInfo(
                        on_wait=[waits[-1]], on_update=list(si.on_update)
                    )
                out.append(inst)
            bb.instructions = out


def _build():
    nc = bass.Bass("TRN2", target_bir_lowering=False, debug=False)
    x = nc.dram_tensor("x_sh", [TOK, C], BF16, kind="ExternalInput").ap()
    wa = nc.dram_tensor("w_attn", [C, 3 * C], BF16, kind="ExternalInput").ap()
    wp = nc.dram_tensor("w_proj", [C, C], BF16, kind="ExternalInput").ap()
    out = nc.dram_tensor("out", [TOK, C], F32, kind="ExternalOutput").ap()

    ident_d = nc.inline_tensor(
        np.eye(128, dtype=np.float32).astype(ml_dtypes.bfloat16), "ident_c"
    ).ap()

    with tile.TileContext(nc) as tc:
        with (
            tc.tile_pool(name="const", bufs=1) as cpool,
            tc.tile_pool(name="wres", bufs=1) as wres,
            tc.tile_pool(name="stream", bufs=1) as stream,
            tc.tile_pool(name="work", bufs=1) as work,
            tc.tile_pool(name="ps", bufs=1, space="PSUM") as pspool,
        ):
            # ---- PSUM layout: separate tiles (sync tracking is per-tile,
            # a shared tile would serialize bank rotation at tile granularity)
            sb = [pspool.tile([128, 512], F32, tag=f"sb{i}", name=f"sb{i}")
                  for i in range(3)]                                     # banks 0-2
            av = [pspool.tile([128, 512], F32, tag=f"av{i}", name=f"av{i}")
                  for i in range(2)]                                     # banks 3-4
            bu0 = pspool.tile([128, 512], F32, tag="bu0", name="bu0")    # bank 5
            atr = pspool.tile([128, 512], BF16, tag="atr", name="atr")   # bank 6
            dtr = pspool.tile([128, 512], BF16, tag="dtr", name="dtr")   # bank 7

            def sbank(i):
                return sb[i][:, :]

            ident = cpool.tile([128, 128], BF16, tag="ident", name="ident")
            nc.sync.dma_start(ident[:], ident_d[:, :])

            # ---- resident weights (bf16, loaded once), wa split per QKV group
            wa_sb = [wres.tile([128, 3 * C], BF16, tag=f"wa{cc}", name=f"wa{cc}")
                     for cc in range(8)]
            wp_sb = [wres.tile([128, C], BF16, tag=f"wp{cc}", name=f"wp{cc}")
                     for cc in range(8)]

            # ---- per-segment working tiles, double-buffered by parity
            xn = [[stream.tile([128, C], BF16, tag=f"xn{p}_{qt}", name=f"xn{p}_{qt}")
                   for qt in range(4)] for p in range(2)]
            xT = [[work.tile([128, SEG], BF16, tag=f"xT{p}_{cc}", name=f"xT{p}_{cc}")
                   for cc in range(8)] for p in range(2)]
            qkt = [[work.tile([128, SEG], BF16, tag=f"qkt{p}_{m}", name=f"qkt{p}_{m}")
                    for m in range(16)] for p in range(2)]
            vp = [[work.tile([128, 16 * 66], BF16, tag=f"vp{p}_{qt}", name=f"vp{p}_{qt}")
                   for qt in range(4)] for p in range(2)]
            ytp = [[work.tile([128, C], BF16, tag=f"ytp{p}_{qc}", name=f"ytp{p}_{qc}")
                    for qc in range(4)] for p in range(2)]
            yt_all = work.tile([128, 4096], BF16, tag="yt", name="yt")

            # ones columns of vp (col 64 of each 66-stride head block) persist
            for p in range(2):
                for qt in range(4):
                    nc.vector.memset(
                        vp[p][qt].rearrange("p (h w) -> p h w", w=66)[:, :, 64:65], 1.0
                    )

            # ---------- emission helpers ----------
            def dma_xn(s):
                p = s % 2
                for qt in range(4):
                    nc.sync.dma_start(
                        xn[p][qt][:],
                        x[s * SEG + qt * 128: s * SEG + (qt + 1) * 128, :],
                    )

            def a_unit(s, cc):
                # x^T for channel chunk cc: 4 transposes into bf16 staging
                p = s % 2
                st = atr if cc % 2 == 0 else dtr
                for qt in range(4):
                    nc.tensor.transpose(
                        st[:, qt * 128:(qt + 1) * 128],
                        xn[p][qt][:, cc * 128:(cc + 1) * 128], ident[:],
                    )
                nc.vector.tensor_copy(xT[p][cc][:], st[:, :])

            def qk_unit(s, u, b):
                # Q^T/K^T channel chunk: g = u//8 (0=Q,1=K), m = u%8
                p = s % 2
                g, m = u // 8, u % 8
                for cc in range(8):
                    nc.tensor.matmul(
                        b[:, :],
                        wa_sb[cc][:, g * C + m * 128: g * C + (m + 1) * 128],
                        xT[p][cc][:],
                        start=(cc == 0), stop=(cc == 7),
                    )
                nc.vector.tensor_copy(qkt[p][g * 8 + m][:], b[:, :])

            def v_unit(s, u, b):
                # V tok-chunk qt = u//2, channel half vn = u%2
                p = s % 2
                qt, vn = u // 2, u % 2
                for cc in range(8):
                    nc.tensor.matmul(
                        b[:, :],
                        xT[p][cc][:, qt * 128:(qt + 1) * 128],
                        wa_sb[cc][:, 2 * C + vn * 512: 2 * C + (vn + 1) * 512],
                        start=(cc == 0), stop=(cc == 7),
                    )
                nc.vector.tensor_copy(
                    vp[p][qt].rearrange("p (h w) -> p h w", w=66)[:, vn * 8:(vn + 1) * 8, 0:64],
                    b.rearrange("p (h w) -> p h w", w=64),
                )

            at0s = {}

            def head_s(s, h):
                # S^T chunks into banks 0-3 + two exp halves -> at0 (bf16)
                p = s % 2
                qk_q = qkt[p][h // 2]
                qk_k = qkt[p][8 + h // 2]
                r0 = (h % 2) * 64
                at0 = at0s[h % 3] = work.tile([128, 2048], BF16, tag="at0",
                                              bufs=3, name=f"at0_{s}_{h}")
                for kt in range(4):
                    b = sbank((h * 4 + kt) % 3)
                    nc.tensor.matmul(
                        b, qk_k[r0:r0 + 64, kt * 128:(kt + 1) * 128],
                        qk_q[r0:r0 + 64, :], start=True, stop=True,
                    )
                    nc.scalar.activation(
                        at0[:, kt * 512:(kt + 1) * 512], b, EXP, scale=SCALE,
                    )

            def head_av(s, h):
                p = s % 2
                at0 = at0s[h % 3]
                a = av[h % 2]
                for qc in range(4):
                    for kt in range(4):
                        nc.tensor.matmul(
                            a[:, qc * 128: qc * 128 + 65],
                            at0[:, kt * 512 + qc * 128: kt * 512 + (qc + 1) * 128],
                            vp[p][kt][:, 66 * h: 66 * h + 65],
                            start=(kt == 0), stop=(kt == 3),
                        )
                zrec = work.tile([128, 4], F32, tag="zrec", bufs=4,
                                 name=f"zrec_{s}_{h}")
                nc.vector.reciprocal(
                    zrec[:, :],
                    a.rearrange("p (q w) -> p q w", w=128)[:, :, 64:65],
                )
                for qc in range(4):
                    nc.vector.tensor_scalar_mul(
                        ytp[p][qc][:, 64 * h: 64 * h + 64],
                        a[:, qc * 128: qc * 128 + 64],
                        zrec[:, qc:qc + 1],
                    )

            ytv = yt_all.rearrange("p (c t) -> p c t", t=512)

            def ytr_unit(s, qc, ccg, eng):
                p = s % 2
                b = atr if (qc * 2 + ccg) % 2 == 0 else dtr
                for j in range(4):
                    cc = ccg * 4 + j
                    nc.tensor.transpose(
                        b[:, j * 128:(j + 1) * 128],
                        ytp[p][qc][:, cc * 128:(cc + 1) * 128], ident[:],
                    )
                eng(
                    ytv[:, ccg * 4:(ccg + 1) * 4, qc * 128:(qc + 1) * 128],
                    b.rearrange("p (c t) -> p c t", t=128),
                )

            def proj_unit(s, m, vn, b, ob, eng):
                for cc in range(8):
                    nc.tensor.matmul(
                        b[:, :],
                        yt_all[:, cc * 512 + m * 128: cc * 512 + (m + 1) * 128],
                        wp_sb[cc][:, vn * 512:(vn + 1) * 512],
                        start=(cc == 0), stop=(cc == 7),
                    )
                eng(ob[:, vn * 512:(vn + 1) * 512], b[:, :])
                rows = out[s * SEG + m * 128: s * SEG + (m + 1) * 128, :]
                if s == NSEG - 1 and m == 3:
                    # split the very last writeback so the vn0 half DMAs
                    # while the vn1 projection is still on the PE
                    nc.sync.dma_start(
                        rows[:, vn * 512:(vn + 1) * 512],
                        ob[:, vn * 512:(vn + 1) * 512],
                    )
                elif vn == 1:
                    nc.sync.dma_start(rows, ob[:])

            def d_units(s, pbanks, eng):
                # interleaved Y^T transpose + projection units for segment s
                units = []
                obs = {}

                def mk_ob(m):
                    obs[m] = work.tile([128, C], F32, tag="ob", bufs=2,
                                       name=f"ob{s}_{m}")
                    return obs[m]

                for qc in range(4):
                    units.append((COST_A, lambda s=s, qc=qc: ytr_unit(s, qc, 0, eng)))
                    units.append((COST_A, lambda s=s, qc=qc: ytr_unit(s, qc, 1, eng)))
                    if qc >= 1:
                        m = qc - 1
                        units.append((COST_D, lambda s=s, m=m: proj_unit(
                            s, m, 0, pbanks[m % len(pbanks)], mk_ob(m), eng)))
                        units.append((COST_D, lambda s=s, m=m: proj_unit(
                            s, m, 1, pbanks[(m + 1) % len(pbanks)], obs[m], eng)))
                units.append((COST_D, lambda s=s: proj_unit(
                    s, 3, 0, pbanks[3 % len(pbanks)], mk_ob(3), eng)))
                units.append((COST_D, lambda s=s: proj_unit(
                    s, 3, 1, pbanks[0 % len(pbanks)], obs[3], eng)))
                return units

            # ---------- program ----------
            # prologue: DMA order matters (one effective DMA pipe):
            # x(seg0) -> Wq -> Wk -> Wv -> x(seg1) -> Wproj
            dma_xn(0)
            for g in range(3):
                for cc in range(8):
                    nc.sync.dma_start(
                        wa_sb[cc][:, g * C:(g + 1) * C],
                        wa[cc * 128:(cc + 1) * 128, g * C:(g + 1) * C],
                    )
                if g == 0:
                    for cc in range(8):
                        a_unit(0, cc)
            dma_xn(1)
            for cc in range(8):
                nc.sync.dma_start(wp_sb[cc][:], wp[cc * 128:(cc + 1) * 128, :])
            # B_0 with A_1 units woven in (keeps the x^T staging drains
            # of segment 1 off the critical path of C_0's early slots)
            for u in range(16):
                qk_unit(0, u, [bu0, av[0]][u % 2])
                if u < 8:
                    a_unit(1, u)
            for u in range(8):
                v_unit(0, u, [bu0, av[0]][u % 2])

            for s in range(NSEG):
                if s + 2 < NSEG:
                    dma_xn(s + 2)
                if s + 1 < NSEG:
                    # B units of s+1; their x^T (A units of s+1) ran during
                    # the previous segment's drain region
                    fillers = [(COST_B, lambda b, s=s, u=u: qk_unit(s + 1, u, b))
                               for u in range(16)]
                    fillers += [(COST_B, lambda b, s=s, u=u: v_unit(s + 1, u, b))
                                for u in range(8)]
                    dpaced = False
                else:
                    fillers = [(c, lambda b, f=f: f())
                               for c, f in d_units(s - 1, [bu0], nc.vector.tensor_copy)]
                    dpaced = True  # exactly one unit per slot: 16 units/16 slots
                for h in range(H):
                    head_s(s, h)
                    if h > 0:
                        head_av(s, h - 1)
                    got = big = n = 0
                    while fillers and got < SLOT_FILL and big < 1 \
                            and not (dpaced and n >= 1):
                        c, f = fillers.pop(0)
                        f(bu0)
                        got += c
                        big += c > 500
                        n += 1
                if fillers:
                    _, f = fillers.pop(0)
                    f(bu0)
                head_av(s, H - 1)
                # drain region: leftover B units of s+1 (alternating banks)
                # interleaved with A units of s+2
                a_next = [lambda s=s, cc=cc: a_unit(s + 2, cc)
                          for cc in range(8)] if s + 2 < NSEG else []
                k = 0
                while fillers or a_next:
                    if fillers:
                        _, f = fillers.pop(0)
                        f([bu0, av[0]][k % 2])
                        k += 1
                    if a_next:
                        a_next.pop(0)()
                if s + 1 < NSEG:
                    if s == NSEG - 2:
                        pass  # D of this segment becomes next C's filler
                    else:
                        for _, f in d_units(s, [sbank(1), sbank(2)], nc.scalar.copy):
                            f()
                else:
                    for _, f in d_units(s, [sbank(1), sbank(2)], nc.scalar.copy):
                        f()

    _split_multi_waits(nc)
    return nc


_NC = None


def kernel(x, w_attn, w_proj, split_sections):
    global _NC
    if _NC is None:
        _NC = _build()
    x = np.asarray(x, dtype=np.float32).astype(ml_dtypes.bfloat16)
    w_attn = np.asarray(w_attn, dtype=np.float32).astype(ml_dtypes.bfloat16)
    w_proj = np.asarray(w_proj, dtype=np.float32).astype(ml_dtypes.bfloat16)
    in_maps = [
        {"x_sh": np.ascontiguousarray(x[i * TOK:(i + 1) * TOK]),
         "w_attn": w_attn, "w_proj": w_proj}
        for i in range(NCORES)
    ]
    res = run_bass_kernel_spmd(_NC, in_maps, core_ids=list(range(NCORES)))
    return np.concatenate([res.results[i]["out"] for i in range(NCORES)], axis=0)


if __name__ == "__main__":
    rng = np.random.default_rng(0)
    x = rng.standard_normal((T, C), dtype=np.float32)
    wa = (rng.standard_normal((C, 3 * C), dtype=np.float32) / np.sqrt(C)).astype(np.float32)
    wpj = (rng.standard_normal((C, C), dtype=np.float32) / np.sqrt(C)).astype(np.float32)
    y = kernel(x, wa, wpj, np.arange(1, 32) * 512)
    print("out", y.shape, y.dtype, np.abs(y).mean())
